# revision 11
# baseline (speedup 1.0000x reference)
"""Trainium2 Bass kernel for stacked per-position FC layer (Conv1d k=1 bank).

Computes out[b, o, i] = sum_c x[b, c, i] * W[i, o, c] + bias[i, o]
for x [64, 256, 2048], W [2048, 256, 256], bias [2048, 256] (fp32).

Strategy: shard positions (2048) across 8 NeuronCores (256 each) —
embarrassingly parallel, no collectives. Per position, one GEMM:
  out_i[b, o] = lhsT.T @ rhs with lhsT = x_i^T [c, b] (stationary),
  rhs = W_i^T [c, o] (moving), contraction c = 256 split in 2 k-tiles
  of 128 partitions, accumulated in PSUM. Bias is added with a K=1
  accumulating matmul against a ones-row.

Host pre-permutes inputs to channel-major / position-middle layouts so
every device DMA is a [128-partition x multi-KB-contiguous-run] pattern
(fp32 has no DMA-transpose path on TRN2):
  x -> [c, i, b]   W -> [c, i, o]   out <- [b, i, o]
"""

import numpy as np

import concourse.bacc as bacc
import concourse.bass as bass
import concourse.mybir as mybir
import concourse.tile as tile
from concourse.bass_utils import run_bass_kernel_spmd

N_CORES = 8
N_POS = 2048
P_LOC = N_POS // N_CORES  # 256 positions per core
C = 256  # contraction (c_in)
B = 64   # batch
O = 256  # c_out
KP = 128  # contraction tile (partition dim)
KT = C // KP  # 2 k-tiles

# Tunables
T = 8                        # positions per DMA tile
MM_DT = mybir.dt.float32     # main matmul dtype (float32 or float32r)
BIAS_DT = mybir.dt.float32   # bias matmul dtype
IO_DT = mybir.dt.float32     # dtype of declared DRAM params / SBUF tiles


def build_program(p_loc=P_LOC, t=T, mm_dt=MM_DT, bias_dt=BIAS_DT, io_dt=IO_DT):
    nc = bacc.Bacc("TRN2", target_bir_lowering=False, debug=False)
    f32 = io_dt
    xt = nc.declare_dram_parameter("xt", [C, p_loc, B], f32, isOutput=False)
    wt = nc.declare_dram_parameter("wt", [C, p_loc, O], f32, isOutput=False)
    bt = nc.declare_dram_parameter("bt", [p_loc, O], f32, isOutput=False)
    out = nc.declare_dram_parameter("out", [B, p_loc, O], mybir.dt.float32,
                                    isOutput=True)

    n_tiles = p_loc // t

    with tile.TileContext(nc) as tc:
        with (
            tc.tile_pool(name="wp", bufs=2 * KT) as w_pool,
            tc.tile_pool(name="xp", bufs=2 * KT) as x_pool,
            tc.tile_pool(name="bp", bufs=2) as b_pool,
            tc.tile_pool(name="op", bufs=3) as o_pool,
            tc.tile_pool(name="cp", bufs=1) as c_pool,
            tc.tile_pool(name="pp", bufs=6, space="PSUM") as ps_pool,
        ):
            ones = c_pool.tile([1, B], f32)
            nc.vector.memset(ones[:, :], 1.0)

            for it in range(n_tiles):
                p0 = it * t
                w_sb = []
                x_sb = []
                for k in range(KT):
                    wk = w_pool.tile([KP, t * O], f32, tag="w")
                    nc.sync.dma_start(
                        out=wk[:, :],
                        in_=wt[k * KP:(k + 1) * KP, p0:p0 + t, :],
                    )
                    w_sb.append(wk)
                    xk = x_pool.tile([KP, t * B], f32, tag="x")
                    nc.sync.dma_start(
                        out=xk[:, :],
                        in_=xt[k * KP:(k + 1) * KP, p0:p0 + t, :],
                    )
                    x_sb.append(xk)
                bsb = b_pool.tile([1, t * O], f32)
                nc.sync.dma_start(out=bsb[0:1, :], in_=bt[p0:p0 + t, :])

                ob = o_pool.tile([B, t * O], mybir.dt.float32)
                for j in range(t):
                    ps = ps_pool.tile([B, O], mybir.dt.float32)
                    for k in range(KT):
                        nc.tensor.matmul(
                            ps[:, :],
                            x_sb[k][:, j * B:(j + 1) * B].bitcast(mm_dt),
                            w_sb[k][:, j * O:(j + 1) * O].bitcast(mm_dt),
                            start=(k == 0),
                            stop=False,
                        )
                    nc.tensor.matmul(
                        ps[:, :],
                        ones[:, :].bitcast(bias_dt),
                        bsb[:, j * O:(j + 1) * O].bitcast(bias_dt),
                        start=False,
                        stop=True,
                    )
                    nc.vector.tensor_copy(ob[:, j * O:(j + 1) * O], ps[:, :])
                nc.sync.dma_start(
                    out=out[:, p0:p0 + t, :],
                    in_=ob[:, :].rearrange("b (t o) -> b t o", t=t),
                )
    nc.compile()
    return nc


def _host_prep(x, W, b):
    """Permute inputs to device layouts; per-core contiguous slices.

    Returns xt8 [8, C, P_LOC, B], wt8 [8, C, P_LOC, O], bt8 [8, P_LOC, O].
    Uses jax on CPU when available (multithreaded transpose), else numpy.
    """
    try:
        import jax
        import jax.numpy as jnp
        cpu = jax.devices("cpu")[0]
        with jax.default_device(cpu):
            xj = jnp.asarray(np.asarray(x, dtype=np.float32))
            wj = jnp.asarray(np.asarray(W, dtype=np.float32))
            # x [B, C, 8*PL] -> [8, C, PL, B]
            xt8 = np.asarray(
                jnp.transpose(xj.reshape(B, C, N_CORES, P_LOC), (2, 1, 3, 0)))
            # W [8*PL, O, C] -> [8, C, PL, O]
            wt8 = np.asarray(
                jnp.transpose(wj.reshape(N_CORES, P_LOC, O, C), (0, 3, 1, 2)))
    except Exception:
        x = np.asarray(x, dtype=np.float32)
        W = np.asarray(W, dtype=np.float32)
        xt8 = np.ascontiguousarray(
            x.reshape(B, C, N_CORES, P_LOC).transpose(2, 1, 3, 0))
        wt8 = np.ascontiguousarray(
            W.reshape(N_CORES, P_LOC, O, C).transpose(0, 3, 1, 2))
    bt8 = np.ascontiguousarray(
        np.asarray(b, dtype=np.float32).reshape(N_CORES, P_LOC, O))
    return xt8, wt8, bt8


def make_in_maps(x, W, b):
    xt8, wt8, bt8 = _host_prep(x, W, b)
    return [{"xt": xt8[d], "wt": wt8[d], "bt": bt8[d]}
            for d in range(N_CORES)]


def run(in_maps, trace=False, **kwargs):
    nc = build_program()
    return run_bass_kernel_spmd(nc, in_maps, list(range(N_CORES)),
                                trace=trace, **kwargs)


def assemble_output(results):
    # results[d]["out"]: [B, P_LOC, O] -> out[b, o, i]
    stacked = np.stack([results[d]["out"] for d in range(N_CORES)])
    # [8, B, PL, O] -> [B, O, 8, PL] -> [B, O, N_POS]
    return np.ascontiguousarray(
        stacked.transpose(1, 3, 0, 2).reshape(B, O, N_POS))


def kernel(x, W, b):
    in_maps = make_in_maps(x, W, b)
    res = run(in_maps)
    return assemble_output(res.results)


# revision 15
# speedup vs baseline: 1.4353x; 1.4353x over previous
"""Trainium2 Bass kernel for stacked per-position FC layer (Conv1d k=1 bank).

Computes out[b, o, i] = sum_c x[b, c, i] * W[i, o, c] + bias[i, o]
for x [64, 256, 2048], W [2048, 256, 256], bias [2048, 256] (fp32).

Strategy: shard positions (2048) across 8 NeuronCores (256 each) —
embarrassingly parallel, no collectives. Per position, one GEMM:
  out_i[b, o] = lhsT.T @ rhs with lhsT = x_i^T [c, b] (stationary),
  rhs = W_i^T [c, o] (moving), contraction c = 256 split in 2 k-tiles
  of 128 partitions, accumulated in PSUM. Bias is added with a K=1
  accumulating matmul against a ones-row.

Host pre-permutes inputs to channel-major / position-middle layouts so
every device DMA is a [128-partition x multi-KB-contiguous-run] pattern
(fp32 has no DMA-transpose path on TRN2):
  x -> [c, i, b]   W -> [c, i, o]   out <- [b, i, o]
"""

import numpy as np

import concourse.bacc as bacc
import concourse.bass as bass
import concourse.mybir as mybir
import concourse.tile as tile
from concourse.bass_utils import run_bass_kernel_spmd

N_CORES = 8
N_POS = 2048
P_LOC = N_POS // N_CORES  # 256 positions per core
C = 256  # contraction (c_in)
B = 64   # batch
O = 256  # c_out
KP = 128  # contraction tile (partition dim)
KT = C // KP  # 2 k-tiles

# Tunables
T = 8                        # positions per DMA tile
MM_DT = mybir.dt.float32r    # main matmul dtype (float32 or float32r)
BIAS_DT = mybir.dt.float32r  # bias matmul dtype
IO_DT = mybir.dt.float32r    # dtype of declared DRAM params / SBUF tiles


def build_program(p_loc=P_LOC, t=T, mm_dt=MM_DT, bias_dt=BIAS_DT, io_dt=IO_DT):
    nc = bacc.Bacc("TRN2", target_bir_lowering=False, debug=False)
    f32 = io_dt
    xt = nc.declare_dram_parameter("xt", [C, p_loc, B], f32, isOutput=False)
    wt = nc.declare_dram_parameter("wt", [C, p_loc, O], f32, isOutput=False)
    bt = nc.declare_dram_parameter("bt", [p_loc, O], f32, isOutput=False)
    ones_d = nc.declare_dram_parameter("ones", [1, B], f32, isOutput=False)
    out = nc.declare_dram_parameter("out", [B, p_loc, O], mybir.dt.float32,
                                    isOutput=True)

    n_tiles = p_loc // t

    with tile.TileContext(nc) as tc:
        with (
            tc.tile_pool(name="wp", bufs=2 * KT) as w_pool,
            tc.tile_pool(name="xp", bufs=2 * KT) as x_pool,
            tc.tile_pool(name="bp", bufs=2) as b_pool,
            tc.tile_pool(name="op", bufs=3) as o_pool,
            tc.tile_pool(name="cp", bufs=1) as c_pool,
            tc.tile_pool(name="pp", bufs=6, space="PSUM") as ps_pool,
        ):
            ones = c_pool.tile([1, B], f32)
            nc.sync.dma_start(out=ones[0:1, :], in_=ones_d[0:1, :])

            for it in range(n_tiles):
                p0 = it * t
                w_sb = []
                x_sb = []
                for k in range(KT):
                    wk = w_pool.tile([KP, t * O], f32, tag="w")
                    nc.sync.dma_start(
                        out=wk[:, :],
                        in_=wt[k * KP:(k + 1) * KP, p0:p0 + t, :],
                    )
                    w_sb.append(wk)
                    xk = x_pool.tile([KP, t * B], f32, tag="x")
                    nc.sync.dma_start(
                        out=xk[:, :],
                        in_=xt[k * KP:(k + 1) * KP, p0:p0 + t, :],
                    )
                    x_sb.append(xk)
                bsb = b_pool.tile([1, t * O], f32)
                nc.sync.dma_start(out=bsb[0:1, :], in_=bt[p0:p0 + t, :])

                ob = o_pool.tile([B, t * O], mybir.dt.float32)
                for j in range(t):
                    ps = ps_pool.tile([B, O], mybir.dt.float32)
                    for k in range(KT):
                        nc.tensor.matmul(
                            ps[:, :],
                            x_sb[k][:, j * B:(j + 1) * B].bitcast(mm_dt),
                            w_sb[k][:, j * O:(j + 1) * O].bitcast(mm_dt),
                            start=(k == 0),
                            stop=False,
                        )
                    nc.tensor.matmul(
                        ps[:, :],
                        ones[:, :].bitcast(bias_dt),
                        bsb[:, j * O:(j + 1) * O].bitcast(bias_dt),
                        start=False,
                        stop=True,
                    )
                    nc.vector.tensor_copy(ob[:, j * O:(j + 1) * O], ps[:, :])
                nc.sync.dma_start(
                    out=out[:, p0:p0 + t, :],
                    in_=ob[:, :].rearrange("b (t o) -> b t o", t=t),
                )
    nc.compile()
    return nc


def _host_prep(x, W, b):
    """Permute inputs to device layouts; per-core contiguous slices.

    Returns xt8 [8, C, P_LOC, B], wt8 [8, C, P_LOC, O], bt8 [8, P_LOC, O].
    Uses jax on CPU when available (multithreaded transpose), else numpy.
    """
    try:
        import jax
        import jax.numpy as jnp
        cpu = jax.devices("cpu")[0]
        with jax.default_device(cpu):
            xj = jnp.asarray(np.asarray(x, dtype=np.float32))
            wj = jnp.asarray(np.asarray(W, dtype=np.float32))
            # x [B, C, 8*PL] -> [8, C, PL, B]
            xt8 = np.asarray(
                jnp.transpose(xj.reshape(B, C, N_CORES, P_LOC), (2, 1, 3, 0)))
            # W [8*PL, O, C] -> [8, C, PL, O]
            wt8 = np.asarray(
                jnp.transpose(wj.reshape(N_CORES, P_LOC, O, C), (0, 3, 1, 2)))
    except Exception:
        x = np.asarray(x, dtype=np.float32)
        W = np.asarray(W, dtype=np.float32)
        xt8 = np.ascontiguousarray(
            x.reshape(B, C, N_CORES, P_LOC).transpose(2, 1, 3, 0))
        wt8 = np.ascontiguousarray(
            W.reshape(N_CORES, P_LOC, O, C).transpose(0, 3, 1, 2))
    bt8 = np.ascontiguousarray(
        np.asarray(b, dtype=np.float32).reshape(N_CORES, P_LOC, O))
    return xt8, wt8, bt8


def make_in_maps(x, W, b):
    xt8, wt8, bt8 = _host_prep(x, W, b)
    ones = np.ones((1, B), np.float32)
    return [{"xt": xt8[d], "wt": wt8[d], "bt": bt8[d], "ones": ones}
            for d in range(N_CORES)]


def run(in_maps, trace=False, **kwargs):
    nc = build_program()
    return run_bass_kernel_spmd(nc, in_maps, list(range(N_CORES)),
                                trace=trace, **kwargs)


def assemble_output(results):
    # results[d]["out"]: [B, P_LOC, O] -> out[b, o, i]
    stacked = np.stack([results[d]["out"] for d in range(N_CORES)])
    # [8, B, PL, O] -> [B, O, 8, PL] -> [B, O, N_POS]
    return np.ascontiguousarray(
        stacked.transpose(1, 3, 0, 2).reshape(B, O, N_POS))


def kernel(x, W, b):
    in_maps = make_in_maps(x, W, b)
    res = run(in_maps)
    return assemble_output(res.results)


# revision 17
# speedup vs baseline: 1.7012x; 1.1853x over previous
"""Trainium2 Bass kernel for stacked per-position FC layer (Conv1d k=1 bank).

Computes out[b, o, i] = sum_c x[b, c, i] * W[i, o, c] + bias[i, o]
for x [64, 256, 2048], W [2048, 256, 256], bias [2048, 256] (fp32).

Strategy: shard positions (2048) across 8 NeuronCores (256 each) —
embarrassingly parallel, no collectives. Per position, one GEMM:
  out_i[b, o] = lhsT.T @ rhs with lhsT = x_i^T [c, b] (stationary),
  rhs = W_i^T [c, o] (moving), contraction c = 256 split in 2 k-tiles
  of 128 partitions, accumulated in PSUM. Bias is added with a K=1
  accumulating matmul against a ones-row.

Host pre-permutes inputs to channel-major / position-middle layouts so
every device DMA is a [128-partition x multi-KB-contiguous-run] pattern
(fp32 has no DMA-transpose path on TRN2):
  x -> [c, i, b]   W -> [c, i, o]   out <- [b, i, o]
"""

import numpy as np

import concourse.bacc as bacc
import concourse.bass as bass
import concourse.mybir as mybir
import concourse.tile as tile
from concourse.bass_utils import run_bass_kernel_spmd

N_CORES = 8
N_POS = 2048
P_LOC = N_POS // N_CORES  # 256 positions per core
C = 256  # contraction (c_in)
B = 64   # batch
O = 256  # c_out
KP = 128  # contraction tile (partition dim)
KT = C // KP  # 2 k-tiles

# Tunables
T = 16                       # positions per DMA tile
MM_DT = mybir.dt.float32r    # main matmul dtype (float32 or float32r)
BIAS_DT = mybir.dt.float32r  # bias matmul dtype
IO_DT = mybir.dt.float32r    # dtype of declared DRAM params / SBUF tiles


def build_program(p_loc=P_LOC, t=T, mm_dt=MM_DT, bias_dt=BIAS_DT, io_dt=IO_DT):
    nc = bacc.Bacc("TRN2", target_bir_lowering=False, debug=False)
    f32 = io_dt
    xt = nc.declare_dram_parameter("xt", [C, p_loc, B], f32, isOutput=False)
    wt = nc.declare_dram_parameter("wt", [C, p_loc, O], f32, isOutput=False)
    bt = nc.declare_dram_parameter("bt", [p_loc, O], f32, isOutput=False)
    ones_d = nc.declare_dram_parameter("ones", [1, B], f32, isOutput=False)
    out = nc.declare_dram_parameter("out", [B, p_loc, O], mybir.dt.float32,
                                    isOutput=True)

    n_tiles = p_loc // t

    with tile.TileContext(nc) as tc:
        with (
            tc.tile_pool(name="wp", bufs=2 * KT) as w_pool,
            tc.tile_pool(name="xp", bufs=2 * KT) as x_pool,
            tc.tile_pool(name="bp", bufs=2) as b_pool,
            tc.tile_pool(name="op", bufs=3) as o_pool,
            tc.tile_pool(name="cp", bufs=1) as c_pool,
            tc.tile_pool(name="pp", bufs=6, space="PSUM") as ps_pool,
        ):
            ones = c_pool.tile([1, B], f32)
            nc.sync.dma_start(out=ones[0:1, :], in_=ones_d[0:1, :])

            for it in range(n_tiles):
                p0 = it * t
                w_sb = []
                x_sb = []
                for k in range(KT):
                    wk = w_pool.tile([KP, t * O], f32, tag="w")
                    nc.sync.dma_start(
                        out=wk[:, :],
                        in_=wt[k * KP:(k + 1) * KP, p0:p0 + t, :],
                    )
                    w_sb.append(wk)
                    xk = x_pool.tile([KP, t * B], f32, tag="x")
                    nc.scalar.dma_start(
                        out=xk[:, :],
                        in_=xt[k * KP:(k + 1) * KP, p0:p0 + t, :],
                    )
                    x_sb.append(xk)
                bsb = b_pool.tile([1, t * O], f32)
                nc.scalar.dma_start(out=bsb[0:1, :], in_=bt[p0:p0 + t, :])

                ob = o_pool.tile([B, t * O], mybir.dt.float32)
                for j in range(t):
                    ps = ps_pool.tile([B, O], mybir.dt.float32)
                    for k in range(KT):
                        nc.tensor.matmul(
                            ps[:, :],
                            x_sb[k][:, j * B:(j + 1) * B].bitcast(mm_dt),
                            w_sb[k][:, j * O:(j + 1) * O].bitcast(mm_dt),
                            start=(k == 0),
                            stop=False,
                        )
                    nc.tensor.matmul(
                        ps[:, :],
                        ones[:, :].bitcast(bias_dt),
                        bsb[:, j * O:(j + 1) * O].bitcast(bias_dt),
                        start=False,
                        stop=True,
                    )
                    nc.vector.tensor_copy(ob[:, j * O:(j + 1) * O], ps[:, :])
                nc.scalar.dma_start(
                    out=out[:, p0:p0 + t, :],
                    in_=ob[:, :].rearrange("b (t o) -> b t o", t=t),
                )
    nc.compile()
    return nc


def _host_prep(x, W, b):
    """Permute inputs to device layouts; per-core contiguous slices.

    Returns xt8 [8, C, P_LOC, B], wt8 [8, C, P_LOC, O], bt8 [8, P_LOC, O].
    Uses jax on CPU when available (multithreaded transpose), else numpy.
    """
    try:
        import jax
        import jax.numpy as jnp
        cpu = jax.devices("cpu")[0]
        with jax.default_device(cpu):
            xj = jnp.asarray(np.asarray(x, dtype=np.float32))
            wj = jnp.asarray(np.asarray(W, dtype=np.float32))
            # x [B, C, 8*PL] -> [8, C, PL, B]
            xt8 = np.asarray(
                jnp.transpose(xj.reshape(B, C, N_CORES, P_LOC), (2, 1, 3, 0)))
            # W [8*PL, O, C] -> [8, C, PL, O]
            wt8 = np.asarray(
                jnp.transpose(wj.reshape(N_CORES, P_LOC, O, C), (0, 3, 1, 2)))
    except Exception:
        x = np.asarray(x, dtype=np.float32)
        W = np.asarray(W, dtype=np.float32)
        xt8 = np.ascontiguousarray(
            x.reshape(B, C, N_CORES, P_LOC).transpose(2, 1, 3, 0))
        wt8 = np.ascontiguousarray(
            W.reshape(N_CORES, P_LOC, O, C).transpose(0, 3, 1, 2))
    bt8 = np.ascontiguousarray(
        np.asarray(b, dtype=np.float32).reshape(N_CORES, P_LOC, O))
    return xt8, wt8, bt8


def make_in_maps(x, W, b):
    xt8, wt8, bt8 = _host_prep(x, W, b)
    ones = np.ones((1, B), np.float32)
    return [{"xt": xt8[d], "wt": wt8[d], "bt": bt8[d], "ones": ones}
            for d in range(N_CORES)]


def run(in_maps, trace=False, **kwargs):
    nc = build_program()
    return run_bass_kernel_spmd(nc, in_maps, list(range(N_CORES)),
                                trace=trace, **kwargs)


def assemble_output(results):
    # results[d]["out"]: [B, P_LOC, O] -> out[b, o, i]
    stacked = np.stack([results[d]["out"] for d in range(N_CORES)])
    # [8, B, PL, O] -> [B, O, 8, PL] -> [B, O, N_POS]
    return np.ascontiguousarray(
        stacked.transpose(1, 3, 0, 2).reshape(B, O, N_POS))


def kernel(x, W, b):
    in_maps = make_in_maps(x, W, b)
    res = run(in_maps)
    return assemble_output(res.results)
